# revision 1
# baseline (speedup 1.0000x reference)
"""DSAutoCorrelation Trainium2 kernel.

Math (verified vs reference, rel err ~8e-7 in numpy):
  C = H*E = 512 channels, L = 2048, B = 16, top_k = 7.
  R[b,l]    = sum_t <k[b,t,:], q[b,(t+l)%L,:]>_c      (= C * mean_value[b,l])
  topk over mean_b R -> 7 delays d_k; w[b,:] = softmax(R[b,d]/C)
  out[b,l,:] = sum_k w[b,k] * v[b,(l+d_k)%L,:]

Device split (8 cores, 2 batches each):
  K1: D[b,p,u] = sum_{i<16, c} K^T[c,128i+p] * Q^T[c,(128i+u)%L]  (pure PE matmul)
      host: R[b,l] = sum_p D[b,p,(p+l)%L]  (tiny reindex) -> topk -> softmax
  K2: rolled weighted sum of V^T via dynamic-offset DVE taps, PE-transpose back
      to natural [l,c] layout, DMA out.
"""

import numpy as np

B, L, H, E = 16, 2048, 8, 64
C = H * E
NCORES = 8
BPC = B // NCORES
TOPK = 7  # int(math.log(2048))
NB = L // 128  # 16 row-blocks

_CACHE = {}


def _f32r():
    from concourse import mybir
    return mybir.dt.float32r


def _build_k1():
    from concourse import bacc, mybir
    from concourse.tile import TileContext

    f32 = mybir.dt.float32
    f32r = mybir.dt.float32r
    nc = bacc.Bacc("TRN2", target_bir_lowering=False, debug=False, num_devices=NCORES)
    qt = nc.dram_tensor("qt", (BPC, C, L), f32r, kind="ExternalInput")
    kt = nc.dram_tensor("kt", (BPC, C, L), f32r, kind="ExternalInput")
    Dout = nc.dram_tensor("D", (BPC, 128, L), f32, kind="ExternalOutput")

    with TileContext(nc) as tc:
        with (
            tc.tile_pool(name="qk", bufs=2) as qkpool,
            tc.tile_pool(name="ps", bufs=2, space="PSUM") as pspool,
            tc.tile_pool(name="dsb", bufs=4) as dpool,
        ):
            for b in range(BPC):
                kts = []
                qts = []
                for cb in range(4):
                    kt_t = qkpool.tile([128, L], f32r, tag=f"kt{cb}", name=f"kt{cb}")
                    nc.sync.dma_start(kt_t[:], kt[b, 128 * cb:128 * (cb + 1), :])
                    kts.append(kt_t)
                    qt_t = qkpool.tile([128, L], f32r, tag=f"qt{cb}", name=f"qt{cb}")
                    nc.sync.dma_start(qt_t[:], qt[b, 128 * cb:128 * (cb + 1), :])
                    qts.append(qt_t)

                psums = [pspool.tile([128, 512], f32, tag=f"ps{u}", name=f"ps{u}") for u in range(4)]
                first = [True] * 4
                for i in range(NB):
                    for cb in range(4):
                        lhs = kts[cb][:, 128 * i:128 * (i + 1)]
                        for u in range(4):
                            u0 = 512 * u
                            s = (128 * i + u0) % L
                            last = (i == NB - 1) and (cb == 3)
                            if s + 512 <= L:
                                nc.tensor.matmul(
                                    psums[u][:, 0:512], lhs, qts[cb][:, s:s + 512],
                                    start=first[u], stop=last)
                            else:
                                n1 = L - s
                                nc.tensor.matmul(
                                    psums[u][:, 0:n1], lhs, qts[cb][:, s:L],
                                    start=first[u], stop=False)
                                nc.tensor.matmul(
                                    psums[u][:, n1:512], lhs, qts[cb][:, 0:512 - n1],
                                    start=first[u], stop=last)
                            first[u] = False
                for u in range(4):
                    d_sb = dpool.tile([128, 512], f32, tag="dsb", name="dsb")
                    nc.vector.tensor_copy(d_sb[:], psums[u][:])
                    nc.sync.dma_start(Dout[b, :, 512 * u:512 * (u + 1)], d_sb[:])
    nc.compile()
    return nc


def _build_k2():
    from concourse import bacc, bass, mybir
    from concourse.tile import TileContext

    f32 = mybir.dt.float32
    i32 = mybir.dt.int32
    nc = bacc.Bacc("TRN2", target_bir_lowering=False, debug=False, num_devices=NCORES)
    vns = [nc.dram_tensor(f"v{b}", (L, C), f32, kind="ExternalInput")
           for b in range(BPC)]
    wb = nc.dram_tensor("wb", (BPC, 128, TOPK), f32, kind="ExternalInput")
    gidx = nc.dram_tensor("gidx", (128, NB * TOPK), i32, kind="ExternalInput")
    out = nc.dram_tensor("out", (BPC, L, C), f32, kind="ExternalOutput")

    with TileContext(nc) as tc:
        with (
            tc.tile_pool(name="consts", bufs=1) as cpool,
            tc.tile_pool(name="taps", bufs=6) as tappool,
            tc.tile_pool(name="acc", bufs=4) as accpool,
        ):
            gi_sb = cpool.tile([128, NB * TOPK], i32, name="gi_sb")
            nc.sync.dma_start(gi_sb[:], gidx[:, :])
            w_sbs = []
            for b in range(BPC):
                w_sb = cpool.tile([128, TOPK], f32, tag=f"w{b}", name=f"w{b}")
                nc.sync.dma_start(w_sb[:], wb[b, :, :])
                w_sbs.append(w_sb)
            for b in range(BPC):
                for m in range(NB):
                    tap = tappool.tile([128, TOPK * C], f32, tag="tap", name="tap")
                    for k in range(TOPK):
                        nc.gpsimd.indirect_dma_start(
                            out=tap[:, C * k:C * (k + 1)],
                            out_offset=None,
                            in_=vns[b][:, :],
                            in_offset=bass.IndirectOffsetOnAxis(
                                ap=gi_sb[:, m * TOPK + k:m * TOPK + k + 1], axis=0),
                        )
                    acc = accpool.tile([128, C], f32, tag="acc", name="acc")
                    nc.vector.tensor_scalar(
                        acc[:], tap[:, 0:C], w_sbs[b][:, 0:1], None,
                        mybir.AluOpType.mult)
                    for k in range(1, TOPK):
                        nc.vector.scalar_tensor_tensor(
                            acc[:], tap[:, C * k:C * (k + 1)],
                            w_sbs[b][:, k:k + 1], acc[:],
                            mybir.AluOpType.mult, mybir.AluOpType.add)
                    nc.sync.dma_start(out[b, 128 * m:128 * (m + 1), :], acc[:])
    nc.compile()
    return nc


def _get_kernels():
    if "k1" not in _CACHE:
        _CACHE["k1"] = _build_k1()
        _CACHE["k2"] = _build_k2()
    return _CACHE["k1"], _CACHE["k2"]


_DIAG_P = np.arange(128)[:, None]
_DIAG_IDX = (np.arange(128)[:, None] + np.arange(L)[None, :]) % L


def kernel(queries, keys, values, attn_mask=None, _trace=False):
    from concourse import bass_utils

    k1, k2 = _get_kernels()
    q = np.ascontiguousarray(np.asarray(queries, dtype=np.float32).reshape(B, L, C).transpose(0, 2, 1))
    kk = np.ascontiguousarray(np.asarray(keys, dtype=np.float32).reshape(B, L, C).transpose(0, 2, 1))
    v = np.ascontiguousarray(np.asarray(values, dtype=np.float32).reshape(B, L, C))

    in1 = [{"qt": q[BPC * r:BPC * (r + 1)], "kt": kk[BPC * r:BPC * (r + 1)]}
           for r in range(NCORES)]
    res1 = bass_utils.run_bass_kernel_spmd(
        k1, in1, core_ids=list(range(NCORES)), trace=_trace)
    D = np.concatenate([r["D"] for r in res1.results], axis=0)  # [B, 128, L]

    R = D[:, _DIAG_P, _DIAG_IDX].sum(axis=1)  # [B, L]
    mean_value = R / C
    didx = np.argsort(-mean_value.mean(axis=0), kind="stable")[:TOPK]
    wlog = mean_value[:, didx]
    wexp = np.exp(wlog - wlog.max(axis=1, keepdims=True))
    w = (wexp / wexp.sum(axis=1, keepdims=True)).astype(np.float32)  # [B, TOPK]

    wb = np.ascontiguousarray(np.repeat(w[:, None, :], 128, axis=1))  # [B,128,TOPK]
    # gidx[p, m*TOPK+k] = (128m + p + d_k) % L
    p_ = np.arange(128)[:, None]
    mk = (128 * (np.arange(NB * TOPK) // TOPK))[None, :] + didx[np.arange(NB * TOPK) % TOPK][None, :]
    gidx = ((p_ + mk) % L).astype(np.int32)
    gidx = np.ascontiguousarray(gidx)
    in2 = [{"v0": v[BPC * r], "v1": v[BPC * r + 1], "wb": wb[BPC * r:BPC * (r + 1)],
            "gidx": gidx} for r in range(NCORES)]
    res2 = bass_utils.run_bass_kernel_spmd(
        k2, in2, core_ids=list(range(NCORES)), trace=_trace)
    out = np.concatenate([r["out"] for r in res2.results], axis=0)  # [B, L, C]
    if _trace:
        kernel._last_trace = (res1, res2)
    return out.reshape(B, L, H, E).astype(np.float32)



# revision 2
# speedup vs baseline: 2.0908x; 2.0908x over previous
"""DSAutoCorrelation Trainium2 kernel (v2).

Math (verified vs reference):
  C = H*E = 512 channels, L = 2048, B = 16, top_k = 7.
  R[b,l]    = sum_t <k[b,t,:], q[b,(t+l)%L,:]>_c      (= C * mean_value[b,l])
  topk over mean_b R -> 7 delays d_k; w[b,:] = softmax(R[b,d]/C)
  out[b,l,:] = sum_k w[b,k] * v[b,(l+d_k)%L,:]

Device split (8 cores, 2 batches each), two launches:
  K1 (compiled once): D[b,p,u] = sum_{i<16,c} K^T[c,128i+p] * Q^T[c,(128i+u)%L]
      via f32r PE matmuls (full rate at N=512; qt padded +512 cols so no
      wrap-split matmuls). D round-trips through DRAM with a diagonal
      read AP to realize R[b,l] = sum_p D[b,p,(p+l)%L]: the diagonal of a
      row-major [128,W] buffer is a linear AP with stride W+1. The
      partition sum runs on gpsimd (partition_all_reduce). Only R
      ([2,1,2048] per core) is downloaded.
  host: topk over batch-mean, softmax -> delays d_k (ints) + weights.
  K2 (compiled per delay-set, cached): the rolled weighted sum runs on
      the PE as transpose-accumulate matmuls: with vt = v^T (bf16,
      [C, L+128] wrap-extended) resident in SBUF,
        psum[l, c-range] += vt[cb][:, 128m+d_k : +128].T @ (w_k * I)
      accumulates over k in PSUM; the matmul simultaneously applies the
      shift (compile-time slice offset), the weight (host-built w_k*I
      moving operand), the tap sum (PSUM accumulate) and the transpose
      back to natural [l, c] layout. No gather DMA, no DVE tap work.
"""

import numpy as np

B, L, H, E = 16, 2048, 8, 64
C = H * E
NCORES = 8
BPC = B // NCORES
TOPK = 7  # int(math.log(2048))
NB = L // 128  # 16 row-blocks
QEXT = L + 512  # wrap-free q window
W = L + 128  # wrap-free vt window / D_ext row length

_CACHE = {}


def _build_k1():
    from concourse import bacc, bass, bass_isa, mybir
    from concourse.tile import TileContext

    f32 = mybir.dt.float32
    f32r = mybir.dt.float32r
    nc = bacc.Bacc("TRN2", target_bir_lowering=False, debug=False, num_devices=NCORES)
    qt = nc.dram_tensor("qt", (BPC, C, L), f32r, kind="ExternalInput")
    kt = nc.dram_tensor("kt", (BPC, C, L), f32r, kind="ExternalInput")
    Rout = nc.dram_tensor("R", (BPC, 1, L), f32, kind="ExternalOutput")

    with TileContext(nc) as tc:
        with (
            tc.tile_pool(name="qk", bufs=3) as qkpool,
            tc.tile_pool(name="ps", bufs=2, space="PSUM") as pspool,
            tc.tile_pool(name="dsb", bufs=2) as dpool,
            tc.tile_pool(name="dext", bufs=2, space="DRAM") as drampool,
        ):
            for b in range(BPC):
                psums = [pspool.tile([128, 512], f32, tag=f"ps{u}", name=f"ps{u}")
                         for u in range(4)]
                for cb in range(4):
                    kt_t = qkpool.tile([128, L], f32r, tag="kt", name="kt_t")
                    nc.sync.dma_start(kt_t[:], kt[b, 128 * cb:128 * (cb + 1), :])
                    qt_t = qkpool.tile([128, QEXT], f32r, tag="qt", name="qt_t")
                    nc.sync.dma_start(qt_t[:, 0:L], qt[b, 128 * cb:128 * (cb + 1), :])
                    nc.sync.dma_start(qt_t[:, L:QEXT],
                                      qt[b, 128 * cb:128 * (cb + 1), 0:512])
                    for i in range(NB):
                        lhs = kt_t[:, 128 * i:128 * (i + 1)]
                        for u in range(4):
                            s = (128 * i + 512 * u) % L
                            nc.tensor.matmul(
                                psums[u][:, 0:512], lhs, qt_t[:, s:s + 512],
                                start=(cb == 0 and i == 0),
                                stop=(cb == 3 and i == NB - 1))
                # D staging: [128, W] with wrap duplicate of first 128 cols
                dstage = dpool.tile([128, W], f32, tag="dstage", name="dstage")
                for u in range(4):
                    nc.vector.tensor_copy(dstage[:, 512 * u:512 * (u + 1)], psums[u][:])
                nc.vector.tensor_copy(dstage[:, L:W], dstage[:, 0:128])
                # DRAM round trip to realize the diagonal access pattern
                dext = drampool.tile([128 * W], f32, tag="dext", name="dext")
                nc.sync.dma_start(dext.rearrange("(p x) -> p x", x=W), dstage[:])
                diag_t = dpool.tile([128, L], f32, tag="diag", name="diag_t")
                diag_ap = bass.AP(dext.tensor, dext.offset, [(W + 1, 128), (1, L)])
                nc.sync.dma_start(diag_t[:], diag_ap)
                # R[b, l] = sum_p diag_t[p, l]  (partition reduce on gpsimd)
                red_t = dpool.tile([128, L], f32, tag="red", name="red_t")
                nc.gpsimd.partition_all_reduce(
                    red_t[:], diag_t[:], 128, bass_isa.ReduceOp.add)
                nc.sync.dma_start(Rout[b, :, :], red_t[0:1, :])
    nc.compile()
    return nc


def _build_k2(delays):
    from concourse import bacc, mybir
    from concourse.tile import TileContext

    f32 = mybir.dt.float32
    bf16 = mybir.dt.bfloat16
    nc = bacc.Bacc("TRN2", target_bir_lowering=False, debug=False, num_devices=NCORES)
    vt = nc.dram_tensor("vt", (BPC, 4, 128, W), bf16, kind="ExternalInput")
    wi = nc.dram_tensor("wi", (BPC, TOPK, 128, 128), bf16, kind="ExternalInput")
    out = nc.dram_tensor("out", (BPC, L, C), bf16, kind="ExternalOutput")

    with TileContext(nc) as tc:
        with (
            tc.tile_pool(name="vpool", bufs=2) as vpool,
            tc.tile_pool(name="cpool", bufs=1) as cpool,
            tc.tile_pool(name="pspool", bufs=6, space="PSUM") as pspool,
            tc.tile_pool(name="opool", bufs=2) as opool,
        ):
            wi_t = cpool.tile([128, BPC * TOPK * 128], bf16, name="wi_t")
            nc.sync.dma_start(wi_t[:], wi[:, :, :, :].transpose([2, 0, 1, 3]))
            for b in range(BPC):
                vts = []
                for cb in range(4):
                    vt_t = vpool.tile([128, W], bf16, tag=f"vt{cb}", name=f"vt{cb}")
                    nc.sync.dma_start(vt_t[:], vt[b, cb, :, :])
                    vts.append(vt_t)
                ostage = opool.tile([128, NB * C], bf16, tag="ostage", name="ostage")
                for m in range(NB):
                    psum = pspool.tile([128, C], f32, tag="ps", name="psum")
                    for cb in range(4):
                        for k in range(TOPK):
                            s = (128 * m + delays[k]) % L
                            nc.tensor.matmul(
                                psum[:, 128 * cb:128 * (cb + 1)],
                                vts[cb][:, s:s + 128],
                                wi_t[:, (b * TOPK + k) * 128:(b * TOPK + k + 1) * 128],
                                start=(k == 0), stop=(k == TOPK - 1))
                    nc.scalar.copy(ostage[:, C * m:C * (m + 1)], psum[:])
                nc.sync.dma_start(
                    out[b, :, :].rearrange("(m p) c -> p m c", p=128), ostage[:])
    nc.compile()
    return nc


def _get_k1():
    if "k1" not in _CACHE:
        _CACHE["k1"] = _build_k1()
    return _CACHE["k1"]


def _get_k2(delays):
    key = ("k2", tuple(int(d) for d in delays))
    if key not in _CACHE:
        _CACHE[key] = _build_k2(tuple(int(d) for d in delays))
    return _CACHE[key]


def kernel(queries, keys, values, attn_mask=None, _trace=False):
    import ml_dtypes
    from concourse import bass_utils

    bf16 = ml_dtypes.bfloat16
    k1 = _get_k1()
    q = np.ascontiguousarray(
        np.asarray(queries, dtype=np.float32).reshape(B, L, C).transpose(0, 2, 1))
    kk = np.ascontiguousarray(
        np.asarray(keys, dtype=np.float32).reshape(B, L, C).transpose(0, 2, 1))
    # vt[b, cb, c, l] = v[b, l, 128*cb + c], wrap-extended to W columns
    v = np.asarray(values, dtype=np.float32).reshape(B, L, C)
    vt = v.reshape(B, L, 4, 128).transpose(0, 2, 3, 1)
    vt = np.concatenate([vt, vt[..., :128]], axis=-1)
    vt = np.ascontiguousarray(vt).astype(bf16)

    in1 = [{"qt": q[BPC * r:BPC * (r + 1)], "kt": kk[BPC * r:BPC * (r + 1)]}
           for r in range(NCORES)]
    res1 = bass_utils.run_bass_kernel_spmd(
        k1, in1, core_ids=list(range(NCORES)), trace=_trace)
    R = np.concatenate([r["R"] for r in res1.results], axis=0)[:, 0, :]  # [B, L]

    mean_value = R / C
    didx = np.argsort(-mean_value.mean(axis=0), kind="stable")[:TOPK]
    wlog = mean_value[:, didx]
    wexp = np.exp(wlog - wlog.max(axis=1, keepdims=True))
    w = (wexp / wexp.sum(axis=1, keepdims=True)).astype(np.float32)  # [B, TOPK]

    # wi[b, k] = w[b, k] * I(128)
    wi = (w[:, :, None, None] * np.eye(128, dtype=np.float32)).astype(bf16)
    wi = np.ascontiguousarray(wi)

    k2 = _get_k2(didx)
    in2 = [{"vt": vt[BPC * r:BPC * (r + 1)], "wi": wi[BPC * r:BPC * (r + 1)]}
           for r in range(NCORES)]
    res2 = bass_utils.run_bass_kernel_spmd(
        k2, in2, core_ids=list(range(NCORES)), trace=_trace)
    out = np.concatenate([r["out"] for r in res2.results], axis=0)  # [B, L, C] bf16
    if _trace:
        kernel._last_trace = (res1, res2)
    return out.astype(np.float32).reshape(B, L, H, E)


# revision 3
# speedup vs baseline: 2.1013x; 1.0050x over previous
"""DSAutoCorrelation Trainium2 kernel (v3).

Math (verified vs reference):
  C = H*E = 512 channels, L = 2048, B = 16, top_k = 7.
  R[b,l]    = sum_t <k[b,t,:], q[b,(t+l)%L,:]>_c      (= C * mean_value[b,l])
  topk over mean_b R -> 7 delays d_k; w[b,:] = softmax(R[b,d]/C)
  out[b,l,:] = sum_k w[b,k] * v[b,(l+d_k)%L,:]

Device split (8 cores, 2 batches each), two launches:
  K1 (compiled once): D[b,p,u] = sum_{i<16,c} K^T[c,128i+p] * Q^T[c,(128i+u)%L]
      via f32r PE matmuls (full rate at N=512; qt padded +512 cols so no
      wrap-split matmuls). D round-trips through DRAM with a diagonal
      read AP to realize R[b,l] = sum_p D[b,p,(p+l)%L]: the diagonal of a
      row-major [128,W] buffer is a linear AP with stride W+1. The
      partition sum is 4 ones-vector matmuls. Only R ([2,1,2048] per
      core) is downloaded. DMA prefetch for phase cb+1 is gated (via
      add_dep_helper) on phase cb's first matmuls so the critical first
      loads don't share DMA bandwidth with prefetches.
  host: topk over batch-mean, softmax -> delays d_k (ints) + weights.
  K2 (compiled per delay-set, cached): the rolled weighted sum runs on
      the PE as transpose-accumulate matmuls: with vt = v^T (bf16,
      [C, L+128] wrap-extended) resident in SBUF,
        psum[l, c-range] += vt[cb][:, 128m+d_k : +128].T @ (w_k * I)
      accumulates over k in PSUM; the matmul simultaneously applies the
      shift (compile-time slice offset), the weight (host-built w_k*I
      moving operand), the tap sum (PSUM accumulate) and the transpose
      back to natural [l, c] layout. No gather DMA, no DVE tap work.
"""

import numpy as np

B, L, H, E = 16, 2048, 8, 64
C = H * E
NCORES = 8
BPC = B // NCORES
TOPK = 7  # int(math.log(2048))
NB = L // 128  # 16 row-blocks
QEXT = L + 512  # wrap-free q window
W = L + 128  # wrap-free vt window / D_ext row length

_CACHE = {}


def _build_k1():
    from concourse import bacc, bass, mybir
    from concourse.tile import TileContext
    from concourse.tile_rust import add_dep_helper

    f32 = mybir.dt.float32
    f32r = mybir.dt.float32r
    nc = bacc.Bacc("TRN2", target_bir_lowering=False, debug=False, num_devices=NCORES)
    qt = nc.dram_tensor("qt", (BPC, C, L), f32r, kind="ExternalInput")
    kt = nc.dram_tensor("kt", (BPC, C, L), f32r, kind="ExternalInput")
    Rout = nc.dram_tensor("R", (BPC, 1, L), f32, kind="ExternalOutput")

    with TileContext(nc) as tc:
        with (
            tc.tile_pool(name="qk", bufs=3) as qkpool,
            tc.tile_pool(name="cpool", bufs=1) as cpool,
            tc.tile_pool(name="ps", bufs=2, space="PSUM") as pspool,
            tc.tile_pool(name="dsb", bufs=2) as dpool,
            tc.tile_pool(name="dext", bufs=2, space="DRAM") as drampool,
        ):
            ones_t = cpool.tile([128, 1], f32, name="ones_t")
            nc.vector.memset(ones_t[:], 1.0)
            gate = None  # first-i-group matmul of previous cb phase
            for b in range(BPC):
                psums = [pspool.tile([128, 512], f32, tag=f"ps{u}", name=f"ps{u}")
                         for u in range(4)]
                for cb in range(4):
                    qt_t = qkpool.tile([128, QEXT], f32r, tag="qt", name="qt_t")
                    kt_t = qkpool.tile([128, L], f32r, tag="kt", name="kt_t")
                    dmas = [
                        nc.sync.dma_start(qt_t[:, 0:L],
                                          qt[b, 128 * cb:128 * (cb + 1), :]),
                        nc.sync.dma_start(qt_t[:, L:QEXT],
                                          qt[b, 128 * cb:128 * (cb + 1), 0:512]),
                        nc.sync.dma_start(kt_t[:], kt[b, 128 * cb:128 * (cb + 1), :]),
                    ]
                    if gate is not None:
                        # keep prefetch DMA off the wire until the previous
                        # phase's compute has started (its loads are done)
                        for d in dmas:
                            add_dep_helper(d.ins, gate.ins, sync=True,
                                           reason="stage prefetch behind compute")
                    for i in range(NB):
                        lhs = kt_t[:, 128 * i:128 * (i + 1)]
                        for u in range(4):
                            s = (128 * i + 512 * u) % L
                            mm = nc.tensor.matmul(
                                psums[u][:, 0:512], lhs, qt_t[:, s:s + 512],
                                start=(cb == 0 and i == 0),
                                stop=(cb == 3 and i == NB - 1))
                            if i == 0 and u == 3:
                                gate = mm
                # D staging: [128, W] with wrap duplicate of first 128 cols
                dstage = dpool.tile([128, W], f32, tag="dstage", name="dstage")
                for u in range(4):
                    nc.vector.tensor_copy(dstage[:, 512 * u:512 * (u + 1)], psums[u][:])
                nc.vector.tensor_copy(dstage[:, L:W], dstage[:, 0:128])
                # DRAM round trip to realize the diagonal access pattern
                dext = drampool.tile([128 * W], f32, tag="dext", name="dext")
                nc.sync.dma_start(dext.rearrange("(p x) -> p x", x=W), dstage[:])
                diag_t = dpool.tile([128, L], f32, tag="diag", name="diag_t")
                diag_ap = bass.AP(dext.tensor, dext.offset, [(W + 1, 128), (1, L)])
                nc.sync.dma_start(diag_t[:], diag_ap)
                # R[b, l] = sum_p diag_t[p, l]: ones-vector matmuls
                r_sb = dpool.tile([1, L], f32, tag="rsb", name="r_sb")
                for u in range(4):
                    ps_r = pspool.tile([128, 512], f32, tag=f"ps{u}", name=f"psr{u}")
                    nc.tensor.matmul(
                        ps_r[0:1, 0:512], ones_t[:].bitcast(f32r),
                        diag_t[:, 512 * u:512 * (u + 1)].bitcast(f32r),
                        start=True, stop=True)
                    nc.scalar.copy(r_sb[0:1, 512 * u:512 * (u + 1)], ps_r[0:1, 0:512])
                nc.sync.dma_start(Rout[b, :, :], r_sb[:])
    nc.compile()
    return nc


def _build_k2(delays):
    from concourse import bacc, mybir
    from concourse.tile import TileContext

    f32 = mybir.dt.float32
    bf16 = mybir.dt.bfloat16
    nc = bacc.Bacc("TRN2", target_bir_lowering=False, debug=False, num_devices=NCORES)
    vt = nc.dram_tensor("vt", (BPC, 4, 128, W), bf16, kind="ExternalInput")
    # wi is host-pretransposed: wi[p, (b*TOPK + k)*128 + c] = w[b,k] * I[p,c]
    wi = nc.dram_tensor("wi", (128, BPC * TOPK * 128), bf16, kind="ExternalInput")
    out = nc.dram_tensor("out", (BPC, L, C), bf16, kind="ExternalOutput")

    with TileContext(nc) as tc:
        with (
            tc.tile_pool(name="vpool", bufs=2) as vpool,
            tc.tile_pool(name="cpool", bufs=1) as cpool,
            tc.tile_pool(name="pspool", bufs=6, space="PSUM") as pspool,
            tc.tile_pool(name="opool", bufs=2) as opool,
        ):
            wi_t = cpool.tile([128, BPC * TOPK * 128], bf16, name="wi_t")
            nc.sync.dma_start(wi_t[:], wi[:, :])
            for b in range(BPC):
                vts = []
                for cb in range(4):
                    vt_t = vpool.tile([128, W], bf16, tag=f"vt{cb}", name=f"vt{cb}")
                    nc.sync.dma_start(vt_t[:], vt[b, cb, :, :])
                    vts.append(vt_t)
                ostage = opool.tile([128, NB * C], bf16, tag="ostage", name="ostage")
                for m in range(NB):
                    psum = pspool.tile([128, C], f32, tag="ps", name="psum")
                    for cb in range(4):
                        for k in range(TOPK):
                            s = (128 * m + delays[k]) % L
                            nc.tensor.matmul(
                                psum[:, 128 * cb:128 * (cb + 1)],
                                vts[cb][:, s:s + 128],
                                wi_t[:, (b * TOPK + k) * 128:(b * TOPK + k + 1) * 128],
                                start=(k == 0), stop=(k == TOPK - 1))
                    nc.scalar.copy(ostage[:, C * m:C * (m + 1)], psum[:])
                    if m % 4 == 3:
                        # stream the finished quarter out so the final DMA
                        # isn't serialized at the kernel tail
                        g = m // 4
                        nc.sync.dma_start(
                            out[b, 512 * g:512 * (g + 1), :].rearrange(
                                "(m p) c -> p m c", p=128),
                            ostage[:, 2048 * g:2048 * (g + 1)])
    nc.compile()
    return nc


def _get_k1():
    if "k1" not in _CACHE:
        _CACHE["k1"] = _build_k1()
    return _CACHE["k1"]


def _get_k2(delays):
    key = ("k2", tuple(int(d) for d in delays))
    if key not in _CACHE:
        _CACHE[key] = _build_k2(tuple(int(d) for d in delays))
    return _CACHE[key]


def kernel(queries, keys, values, attn_mask=None, _trace=False):
    import ml_dtypes
    from concourse import bass_utils

    bf16 = ml_dtypes.bfloat16
    k1 = _get_k1()
    q = np.ascontiguousarray(
        np.asarray(queries, dtype=np.float32).reshape(B, L, C).transpose(0, 2, 1))
    kk = np.ascontiguousarray(
        np.asarray(keys, dtype=np.float32).reshape(B, L, C).transpose(0, 2, 1))
    # vt[b, cb, c, l] = v[b, l, 128*cb + c], wrap-extended to W columns
    v = np.asarray(values, dtype=np.float32).reshape(B, L, C)
    vt = v.reshape(B, L, 4, 128).transpose(0, 2, 3, 1)
    vt = np.concatenate([vt, vt[..., :128]], axis=-1)
    vt = np.ascontiguousarray(vt).astype(bf16)

    in1 = [{"qt": q[BPC * r:BPC * (r + 1)], "kt": kk[BPC * r:BPC * (r + 1)]}
           for r in range(NCORES)]
    res1 = bass_utils.run_bass_kernel_spmd(
        k1, in1, core_ids=list(range(NCORES)), trace=_trace)
    R = np.concatenate([r["R"] for r in res1.results], axis=0)[:, 0, :]  # [B, L]

    mean_value = R / C
    didx = np.argsort(-mean_value.mean(axis=0), kind="stable")[:TOPK]
    wlog = mean_value[:, didx]
    wexp = np.exp(wlog - wlog.max(axis=1, keepdims=True))
    w = (wexp / wexp.sum(axis=1, keepdims=True)).astype(np.float32)  # [B, TOPK]

    # wi[p, (b*TOPK+k)*128 + c] = w[b,k] * I[p,c]  (pretransposed for fast DMA)
    wi = (w[:, :, None, None] * np.eye(128, dtype=np.float32))  # [B,TOPK,128,128]
    wi_p = {}
    for r in range(NCORES):
        blk = wi[BPC * r:BPC * (r + 1)].transpose(2, 0, 1, 3)  # [128,BPC,TOPK,128]
        wi_p[r] = np.ascontiguousarray(blk.reshape(128, BPC * TOPK * 128)).astype(bf16)

    k2 = _get_k2(didx)
    in2 = [{"vt": vt[BPC * r:BPC * (r + 1)], "wi": wi_p[r]} for r in range(NCORES)]
    res2 = bass_utils.run_bass_kernel_spmd(
        k2, in2, core_ids=list(range(NCORES)), trace=_trace)
    out = np.concatenate([r["out"] for r in res2.results], axis=0)  # [B, L, C] bf16
    if _trace:
        kernel._last_trace = (res1, res2)
    return out.astype(np.float32).reshape(B, L, H, E)


# revision 4
# speedup vs baseline: 2.4505x; 1.1662x over previous
"""DSAutoCorrelation Trainium2 kernel (v4).

Math (verified vs reference):
  C = H*E = 512 channels, L = 2048, B = 16, top_k = 7.
  R[b,l]    = sum_t <k[b,t,:], q[b,(t+l)%L,:]>_c      (= C * mean_value[b,l])
  topk over mean_b R -> 7 delays d_k; w[b,:] = softmax(R[b,d]/C)
  out[b,l,:] = sum_k w[b,k] * v[b,(l+d_k)%L,:]

Device split (8 cores, 2 batches each), two launches:
  K1 (compiled once): D[b,p,u] = sum_{i<16,c} K^T[c,128i+p] * Q^T[c,(128i+u)%L]
      via bf16 PE matmuls (full rate, FWL weight loads; qt padded +512
      cols so no wrap-split matmuls; bf16 selection verified against the
      fp32 reference: identical top-7, weight delta < 0.5%). Host gets D
      and does the tiny diagonal reindex R[b,l] = sum_p D[b,p,(p+l)%L],
      topk and softmax. Prefetch DMAs for phase cb+1 are gated (via
      add_dep_helper) on phase cb's first matmuls so the critical first
      loads don't share DMA bandwidth with prefetches.
  K2 (compiled per delay-set, cached): the rolled weighted sum runs on
      the PE as transpose-accumulate matmuls: with vt = v^T (bf16,
      [C, L+128] wrap-extended) resident in SBUF,
        psum[l, c-range] += vt[cb][:, 128m+d_k : +128].T @ (w_k * I)
      accumulates over k in PSUM; the matmul simultaneously applies the
      shift (compile-time slice offset), the weight (host-built w_k*I
      moving operand), the tap sum (PSUM accumulate) and the transpose
      back to natural [l, c] layout. No gather DMA, no DVE tap work.
"""

import numpy as np

B, L, H, E = 16, 2048, 8, 64
C = H * E
NCORES = 8
BPC = B // NCORES
TOPK = 7  # int(math.log(2048))
NB = L // 128  # 16 row-blocks
QEXT = L + 512  # wrap-free q window
W = L + 128  # wrap-free vt window

_CACHE = {}


def _build_k1():
    from concourse import bacc, mybir
    from concourse.tile import TileContext
    from concourse.tile_rust import add_dep_helper

    f32 = mybir.dt.float32
    bf16 = mybir.dt.bfloat16
    nc = bacc.Bacc("TRN2", target_bir_lowering=False, debug=False, num_devices=NCORES)
    qt = nc.dram_tensor("qt", (BPC, C, L), bf16, kind="ExternalInput")
    kt = nc.dram_tensor("kt", (BPC, C, L), bf16, kind="ExternalInput")
    Dout = nc.dram_tensor("D", (BPC, 128, L), f32, kind="ExternalOutput")

    with TileContext(nc) as tc:
        with (
            tc.tile_pool(name="qk", bufs=3) as qkpool,
            tc.tile_pool(name="ps", bufs=2, space="PSUM") as pspool,
            tc.tile_pool(name="dsb", bufs=2) as dpool,
        ):
            gate = None  # first-i-group matmul of previous cb phase
            for b in range(BPC):
                psums = [pspool.tile([128, 512], f32, tag=f"ps{u}", name=f"ps{u}")
                         for u in range(4)]
                for cb in range(4):
                    qt_t = qkpool.tile([128, QEXT], bf16, tag="qt", name="qt_t")
                    kt_t = qkpool.tile([128, L], bf16, tag="kt", name="kt_t")
                    dmas = [
                        nc.sync.dma_start(qt_t[:, 0:L],
                                          qt[b, 128 * cb:128 * (cb + 1), :]),
                        nc.sync.dma_start(qt_t[:, L:QEXT],
                                          qt[b, 128 * cb:128 * (cb + 1), 0:512]),
                    ] + [
                        nc.sync.dma_start(
                            kt_t[:, 512 * j:512 * (j + 1)],
                            kt[b, 128 * cb:128 * (cb + 1), 512 * j:512 * (j + 1)])
                        for j in range(4)
                    ]
                    if gate is not None:
                        # keep prefetch DMA off the wire until the previous
                        # phase's compute has started (its loads are done)
                        for d in dmas:
                            add_dep_helper(d.ins, gate.ins, sync=True,
                                           reason="stage prefetch behind compute")
                    for i in range(NB):
                        lhs = kt_t[:, 128 * i:128 * (i + 1)]
                        for u in range(4):
                            s = (128 * i + 512 * u) % L
                            mm = nc.tensor.matmul(
                                psums[u][:, 0:512], lhs, qt_t[:, s:s + 512],
                                start=(cb == 0 and i == 0),
                                stop=(cb == 3 and i == NB - 1))
                            if i == 0 and u == 3:
                                gate = mm
                dstage = dpool.tile([128, L], f32, tag="dstage", name="dstage")
                for u in range(4):
                    nc.vector.tensor_copy(dstage[:, 512 * u:512 * (u + 1)], psums[u][:])
                nc.sync.dma_start(Dout[b, :, :], dstage[:])
    nc.compile()
    return nc


def _build_k2(delays):
    from concourse import bacc, mybir
    from concourse.tile import TileContext
    from concourse.tile_rust import add_dep_helper

    f32 = mybir.dt.float32
    bf16 = mybir.dt.bfloat16
    nc = bacc.Bacc("TRN2", target_bir_lowering=False, debug=False, num_devices=NCORES)
    vt = nc.dram_tensor("vt", (BPC, 4, 128, W), bf16, kind="ExternalInput")
    # wi is host-pretransposed: wi[p, (b*TOPK + k)*128 + c] = w[b,k] * I[p,c]
    wi = nc.dram_tensor("wi", (128, BPC * TOPK * 128), bf16, kind="ExternalInput")
    out = nc.dram_tensor("out", (BPC, L, C), bf16, kind="ExternalOutput")

    with TileContext(nc) as tc:
        with (
            tc.tile_pool(name="vpool", bufs=2) as vpool,
            tc.tile_pool(name="cpool", bufs=1) as cpool,
            tc.tile_pool(name="pspool", bufs=6, space="PSUM") as pspool,
            tc.tile_pool(name="opool", bufs=2) as opool,
        ):
            wi_t = cpool.tile([128, BPC * TOPK * 128], bf16, name="wi_t")
            nc.sync.dma_start(wi_t[:], wi[:, :])
            gate = None
            for b in range(BPC):
                vts = []
                for cb in range(4):
                    vt_t = vpool.tile([128, W], bf16, tag=f"vt{cb}", name=f"vt{cb}")
                    d = nc.sync.dma_start(vt_t[:], vt[b, cb, :, :])
                    if gate is not None:
                        add_dep_helper(d.ins, gate.ins, sync=True,
                                       reason="stage prefetch behind compute")
                    vts.append(vt_t)
                ostage = opool.tile([128, NB * C], bf16, tag="ostage", name="ostage")
                for m in range(NB):
                    psum = pspool.tile([128, C], f32, tag="ps", name="psum")
                    for cb in range(4):
                        for k in range(TOPK):
                            s = (128 * m + delays[k]) % L
                            mm = nc.tensor.matmul(
                                psum[:, 128 * cb:128 * (cb + 1)],
                                vts[cb][:, s:s + 128],
                                wi_t[:, (b * TOPK + k) * 128:(b * TOPK + k + 1) * 128],
                                start=(k == 0), stop=(k == TOPK - 1))
                            if m == 0 and cb == 3 and k == 0:
                                gate = mm
                    nc.scalar.copy(ostage[:, C * m:C * (m + 1)], psum[:])
                    if m % 4 == 3:
                        # stream the finished quarter out so the final DMA
                        # isn't serialized at the kernel tail
                        g = m // 4
                        nc.sync.dma_start(
                            out[b, 512 * g:512 * (g + 1), :].rearrange(
                                "(m p) c -> p m c", p=128),
                            ostage[:, 2048 * g:2048 * (g + 1)])
    nc.compile()
    return nc


def _get_k1():
    if "k1" not in _CACHE:
        _CACHE["k1"] = _build_k1()
    return _CACHE["k1"]


def _get_k2(delays):
    key = ("k2", tuple(int(d) for d in delays))
    if key not in _CACHE:
        _CACHE[key] = _build_k2(tuple(int(d) for d in delays))
    return _CACHE[key]


_DIAG_P = np.arange(128)[:, None]
_DIAG_IDX = (np.arange(128)[:, None] + np.arange(L)[None, :]) % L


def kernel(queries, keys, values, attn_mask=None, _trace=False):
    import ml_dtypes
    from concourse import bass_utils

    bf16 = ml_dtypes.bfloat16
    k1 = _get_k1()
    q = np.ascontiguousarray(
        np.asarray(queries, dtype=np.float32).reshape(B, L, C).transpose(0, 2, 1)
    ).astype(bf16)
    kk = np.ascontiguousarray(
        np.asarray(keys, dtype=np.float32).reshape(B, L, C).transpose(0, 2, 1)
    ).astype(bf16)
    # vt[b, cb, c, l] = v[b, l, 128*cb + c], wrap-extended to W columns
    v = np.asarray(values, dtype=np.float32).reshape(B, L, C)
    vt = v.reshape(B, L, 4, 128).transpose(0, 2, 3, 1)
    vt = np.concatenate([vt, vt[..., :128]], axis=-1)
    vt = np.ascontiguousarray(vt).astype(bf16)

    in1 = [{"qt": q[BPC * r:BPC * (r + 1)], "kt": kk[BPC * r:BPC * (r + 1)]}
           for r in range(NCORES)]
    res1 = bass_utils.run_bass_kernel_spmd(
        k1, in1, core_ids=list(range(NCORES)), trace=_trace)
    D = np.concatenate([r["D"] for r in res1.results], axis=0)  # [B, 128, L]
    R = D[:, _DIAG_P, _DIAG_IDX].sum(axis=1)  # [B, L]

    mean_value = R / C
    didx = np.argsort(-mean_value.mean(axis=0), kind="stable")[:TOPK]
    wlog = mean_value[:, didx]
    wexp = np.exp(wlog - wlog.max(axis=1, keepdims=True))
    w = (wexp / wexp.sum(axis=1, keepdims=True)).astype(np.float32)  # [B, TOPK]

    # wi[p, (b*TOPK+k)*128 + c] = w[b,k] * I[p,c]  (pretransposed for fast DMA)
    wi = (w[:, :, None, None] * np.eye(128, dtype=np.float32))  # [B,TOPK,128,128]
    wi_p = {}
    for r in range(NCORES):
        blk = wi[BPC * r:BPC * (r + 1)].transpose(2, 0, 1, 3)  # [128,BPC,TOPK,128]
        wi_p[r] = np.ascontiguousarray(blk.reshape(128, BPC * TOPK * 128)).astype(bf16)

    k2 = _get_k2(didx)
    in2 = [{"vt": vt[BPC * r:BPC * (r + 1)], "wi": wi_p[r]} for r in range(NCORES)]
    res2 = bass_utils.run_bass_kernel_spmd(
        k2, in2, core_ids=list(range(NCORES)), trace=_trace)
    out = np.concatenate([r["out"] for r in res2.results], axis=0)  # [B, L, C] bf16
    if _trace:
        kernel._last_trace = (res1, res2)
    return out.astype(np.float32).reshape(B, L, H, E)


# revision 9
# speedup vs baseline: 2.4745x; 1.0098x over previous
"""DSAutoCorrelation Trainium2 kernel (v4).

Math (verified vs reference):
  C = H*E = 512 channels, L = 2048, B = 16, top_k = 7.
  R[b,l]    = sum_t <k[b,t,:], q[b,(t+l)%L,:]>_c      (= C * mean_value[b,l])
  topk over mean_b R -> 7 delays d_k; w[b,:] = softmax(R[b,d]/C)
  out[b,l,:] = sum_k w[b,k] * v[b,(l+d_k)%L,:]

Device split (8 cores, 2 batches each), two launches:
  K1 (compiled once): D[b,p,u] = sum_{i<16,c} K^T[c,128i+p] * Q^T[c,(128i+u)%L]
      via bf16 PE matmuls (full rate, FWL weight loads; qt padded +512
      cols so no wrap-split matmuls; bf16 selection verified against the
      fp32 reference: identical top-7, weight delta < 0.5%). Host gets D
      and does the tiny diagonal reindex R[b,l] = sum_p D[b,p,(p+l)%L],
      topk and softmax. Prefetch DMAs for phase cb+1 are gated (via
      add_dep_helper) on phase cb's first matmuls so the critical first
      loads don't share DMA bandwidth with prefetches.
  K2 (compiled per delay-set, cached): the rolled weighted sum runs on
      the PE as transpose-accumulate matmuls: with vt = v^T (bf16,
      [C, L+128] wrap-extended) resident in SBUF,
        psum[l, c-range] += vt[cb][:, 128m+d_k : +128].T @ (w_k * I)
      accumulates over k in PSUM; the matmul simultaneously applies the
      shift (compile-time slice offset), the weight (host-built w_k*I
      moving operand), the tap sum (PSUM accumulate) and the transpose
      back to natural [l, c] layout. No gather DMA, no DVE tap work.
"""

import numpy as np

B, L, H, E = 16, 2048, 8, 64
C = H * E
NCORES = 8
BPC = B // NCORES
TOPK = 7  # int(math.log(2048))
NB = L // 128  # 16 row-blocks
QEXT = L + 512  # wrap-free q window
W = L + 128  # wrap-free vt window

_CACHE = {}


def _build_k1():
    from concourse import bacc, mybir
    from concourse.tile import TileContext
    from concourse.tile_rust import add_dep_helper

    f32 = mybir.dt.float32
    bf16 = mybir.dt.bfloat16
    nc = bacc.Bacc("TRN2", target_bir_lowering=False, debug=False, num_devices=NCORES)
    qt = nc.dram_tensor("qt", (BPC, C, L), bf16, kind="ExternalInput")
    kt = nc.dram_tensor("kt", (BPC, C, L), bf16, kind="ExternalInput")
    Dout = nc.dram_tensor("D", (BPC, 128, L), f32, kind="ExternalOutput")

    with TileContext(nc) as tc:
        with (
            tc.tile_pool(name="qk", bufs=3) as qkpool,
            tc.tile_pool(name="ps", bufs=2, space="PSUM") as pspool,
            tc.tile_pool(name="dsb", bufs=2) as dpool,
        ):
            gate = None  # first-i-group matmul of previous cb phase
            for b in range(BPC):
                psums = [pspool.tile([128, 512], f32, tag=f"ps{u}", name=f"ps{u}")
                         for u in range(4)]
                for cb in range(4):
                    qt_t = qkpool.tile([128, QEXT], bf16, tag="qt", name="qt_t")
                    kt_t = qkpool.tile([128, L], bf16, tag="kt", name="kt_t")
                    dmas = [
                        nc.sync.dma_start(qt_t[:, 0:L],
                                          qt[b, 128 * cb:128 * (cb + 1), :]),
                        nc.sync.dma_start(kt_t[:], kt[b, 128 * cb:128 * (cb + 1), :]),
                        nc.sync.dma_start(qt_t[:, L:QEXT],
                                          qt[b, 128 * cb:128 * (cb + 1), 0:512]),
                    ]
                    if gate is not None:
                        # keep prefetch DMA off the wire until the previous
                        # phase's compute has started (its loads are done)
                        for d in dmas:
                            add_dep_helper(d.ins, gate.ins, sync=True,
                                           reason="stage prefetch behind compute")
                    for i in range(NB):
                        lhs = kt_t[:, 128 * i:128 * (i + 1)]
                        for u in range(4):
                            s = (128 * i + 512 * u) % L
                            mm = nc.tensor.matmul(
                                psums[u][:, 0:512], lhs, qt_t[:, s:s + 512],
                                start=(cb == 0 and i == 0),
                                stop=(cb == 3 and i == NB - 1))
                            if i == 0 and u == 3:
                                gate = mm
                dstage = dpool.tile([128, L], f32, tag="dstage", name="dstage")
                for u in range(4):
                    nc.vector.tensor_copy(dstage[:, 512 * u:512 * (u + 1)], psums[u][:])
                    if u % 2 == 1:
                        # stream each finished half out so the final DMA
                        # completion isn't fully serialized at the tail
                        nc.sync.dma_start(Dout[b, :, 512 * (u - 1):512 * (u + 1)],
                                          dstage[:, 512 * (u - 1):512 * (u + 1)])
    nc.compile()
    return nc


def _build_k2(delays):
    from concourse import bacc, mybir
    from concourse.tile import TileContext
    from concourse.tile_rust import add_dep_helper

    f32 = mybir.dt.float32
    bf16 = mybir.dt.bfloat16
    nc = bacc.Bacc("TRN2", target_bir_lowering=False, debug=False, num_devices=NCORES)
    vt = nc.dram_tensor("vt", (BPC, 4, 128, W), bf16, kind="ExternalInput")
    # wi is host-pretransposed: wi[p, (b*TOPK + k)*128 + c] = w[b,k] * I[p,c]
    wi = nc.dram_tensor("wi", (128, BPC * TOPK * 128), bf16, kind="ExternalInput")
    out = nc.dram_tensor("out", (BPC, L, C), bf16, kind="ExternalOutput")

    with TileContext(nc) as tc:
        with (
            tc.tile_pool(name="vpool", bufs=2) as vpool,
            tc.tile_pool(name="cpool", bufs=1) as cpool,
            tc.tile_pool(name="pspool", bufs=1, space="PSUM") as pspool,
            tc.tile_pool(name="opool", bufs=2) as opool,
        ):
            wi_t = cpool.tile([128, BPC * TOPK * 128], bf16, name="wi_t")
            nc.sync.dma_start(wi_t[:], wi[:, :])
            gate = None  # staging gate for later vt loads
            for b in range(BPC):
                vts = []
                pending = []  # b=0: cb>0 loads staged once the first mm exists
                for cb in range(4):
                    vt_t = vpool.tile([128, W], bf16, tag=f"vt{cb}", name=f"vt{cb}")
                    d = nc.sync.dma_start(vt_t[:], vt[b, cb, :, :])
                    if gate is not None:
                        add_dep_helper(d.ins, gate.ins, sync=True,
                                       reason="stage prefetch behind compute")
                    elif cb > 0:
                        pending.append(d)
                    vts.append(vt_t)
                ostage = opool.tile([128, NB * C], bf16, tag="ostage", name="ostage")
                # halves of 8 m-blocks, cb-outer: matmuls start after vt[cb=0]
                # alone has landed, instead of all four tiles
                for half in range(2):
                    ms = list(range(8 * half, 8 * half + 8))
                    pss = {m: pspool.tile([128, C], f32, tag=f"ps{m % 8}",
                                          name=f"ps{m % 8}") for m in ms}
                    for cb in range(4):
                        for m in ms:
                            for k in range(TOPK):
                                s = (128 * m + delays[k]) % L
                                mm = nc.tensor.matmul(
                                    pss[m][:, 128 * cb:128 * (cb + 1)],
                                    vts[cb][:, s:s + 128],
                                    wi_t[:, (b * TOPK + k) * 128:
                                            (b * TOPK + k + 1) * 128],
                                    start=(k == 0), stop=(k == TOPK - 1))
                                if m == ms[0] and k == 0 and cb == 0:
                                    if pending:
                                        for d in pending:
                                            add_dep_helper(
                                                d.ins, mm.ins, sync=True,
                                                reason="stage prefetch")
                                        pending = []
                                    if half == 1:
                                        gate = mm
                    for m in ms:
                        nc.scalar.copy(ostage[:, C * m:C * (m + 1)], pss[m][:])
                        if m % 4 == 3:
                            # stream finished quarters out early
                            g = m // 4
                            nc.sync.dma_start(
                                out[b, 512 * g:512 * (g + 1), :].rearrange(
                                    "(m p) c -> p m c", p=128),
                                ostage[:, 2048 * g:2048 * (g + 1)])
    nc.compile()
    return nc


def _get_k1():
    if "k1" not in _CACHE:
        _CACHE["k1"] = _build_k1()
    return _CACHE["k1"]


def _get_k2(delays):
    key = ("k2", tuple(int(d) for d in delays))
    if key not in _CACHE:
        _CACHE[key] = _build_k2(tuple(int(d) for d in delays))
    return _CACHE[key]


_DIAG_P = np.arange(128)[:, None]
_DIAG_IDX = (np.arange(128)[:, None] + np.arange(L)[None, :]) % L


def kernel(queries, keys, values, attn_mask=None, _trace=False):
    import ml_dtypes
    from concourse import bass_utils

    bf16 = ml_dtypes.bfloat16
    k1 = _get_k1()
    q = np.ascontiguousarray(
        np.asarray(queries, dtype=np.float32).reshape(B, L, C).transpose(0, 2, 1)
    ).astype(bf16)
    kk = np.ascontiguousarray(
        np.asarray(keys, dtype=np.float32).reshape(B, L, C).transpose(0, 2, 1)
    ).astype(bf16)
    # vt[b, cb, c, l] = v[b, l, 128*cb + c], wrap-extended to W columns
    v = np.asarray(values, dtype=np.float32).reshape(B, L, C)
    vt = v.reshape(B, L, 4, 128).transpose(0, 2, 3, 1)
    vt = np.concatenate([vt, vt[..., :128]], axis=-1)
    vt = np.ascontiguousarray(vt).astype(bf16)

    in1 = [{"qt": q[BPC * r:BPC * (r + 1)], "kt": kk[BPC * r:BPC * (r + 1)]}
           for r in range(NCORES)]
    res1 = bass_utils.run_bass_kernel_spmd(
        k1, in1, core_ids=list(range(NCORES)), trace=_trace)
    D = np.concatenate([r["D"] for r in res1.results], axis=0)  # [B, 128, L]
    R = D[:, _DIAG_P, _DIAG_IDX].sum(axis=1)  # [B, L]

    mean_value = R / C
    didx = np.argsort(-mean_value.mean(axis=0), kind="stable")[:TOPK]
    wlog = mean_value[:, didx]
    wexp = np.exp(wlog - wlog.max(axis=1, keepdims=True))
    w = (wexp / wexp.sum(axis=1, keepdims=True)).astype(np.float32)  # [B, TOPK]

    # wi[p, (b*TOPK+k)*128 + c] = w[b,k] * I[p,c]  (pretransposed for fast DMA)
    wi = (w[:, :, None, None] * np.eye(128, dtype=np.float32))  # [B,TOPK,128,128]
    wi_p = {}
    for r in range(NCORES):
        blk = wi[BPC * r:BPC * (r + 1)].transpose(2, 0, 1, 3)  # [128,BPC,TOPK,128]
        wi_p[r] = np.ascontiguousarray(blk.reshape(128, BPC * TOPK * 128)).astype(bf16)

    k2 = _get_k2(didx)
    in2 = [{"vt": vt[BPC * r:BPC * (r + 1)], "wi": wi_p[r]} for r in range(NCORES)]
    res2 = bass_utils.run_bass_kernel_spmd(
        k2, in2, core_ids=list(range(NCORES)), trace=_trace)
    out = np.concatenate([r["out"] for r in res2.results], axis=0)  # [B, L, C] bf16
    if _trace:
        kernel._last_trace = (res1, res2)
    return out.astype(np.float32).reshape(B, L, H, E)
